# revision 26
# baseline (speedup 1.0000x reference)
"""Trainium2 Bass kernel for nn_DecoderAMRPALayer (B=2, S=2048, E=2048, d_k=128).

Sharding: 8 cores = 2 batches x 4 row-chunks of 512 query rows. Each core's
hidden input is row-rotated so its 512 local rows come first; the attention
key/value axis is then a (consistent) permutation of positions, which softmax
and the j-contractions are invariant to.

Per-core math (thin-chain reformulation; K/V never materialized):
  Q   = hid[:512] @ Wq + bq                  (K-bias cancels in softmax;
  P^T = Wk^T @ Q^T                            V-bias folds into output bias)
  scores = P @ hid^T
  baseA = softmax(SCALE * Qcam @ K_cam^T)     (K_cam^T, V_cam from host)
  camctx^T = V_cam blocks @ baseA^T tiles
  T^T = lw * tanh(gate*camctx^T + gate*bv_cam)
  logits = SCALE * (scores + (T^T)^T-contraction with K_cam^T)
  A = softmax(logits)
  H^T = hid^T @ A^T ; ctxu^T = Wv^T @ H^T ; out = ctxu @ Wp + (bv@Wp + bp)

All data tensors bf16 (psum f32). hid^T, K_cam^T, V_cam arrive precomputed
from the host (batch-shared tensors; avoids on-device transposes and the 4x
per-batch recompute). P^T / baseA^T / A^T are SBUF-resident (no DRAM spills).
PSUM runs as 6 independent [128,512] accumulator banks (cross-iteration
double buffering) + 2 transpose banks.
"""

import sys

sys.path.insert(0, "/opt/trn_rl_repo")

import numpy as np

import concourse.bass as bass
import concourse.mybir as mybir
from concourse import bacc
from concourse.bass import ts
from concourse.bass_utils import run_bass_kernel_spmd
from concourse.masks import make_identity
from concourse.tile import TileContext

F32 = mybir.dt.float32
BF16 = mybir.dt.bfloat16
AF = mybir.ActivationFunctionType
ALU = mybir.AluOpType

S = 2048
E = 2048
LOC = 512  # local query rows per core
DK = 128
NT = E // 128  # 16 partition tiles
SCALE = 1.0 / float(np.sqrt(128.0))
P = 128


def build():
    nc = bacc.Bacc("TRN2", target_bir_lowering=False, debug=False)

    hidT_d = nc.dram_tensor("hidT", [E, S], BF16, kind="ExternalInput").ap()
    hid = nc.dram_tensor("hid", [S, E], BF16, kind="ExternalInput").ap()
    wq = nc.dram_tensor("wq", [E, E], BF16, kind="ExternalInput").ap()
    wkT = nc.dram_tensor("wkT", [E, E], BF16, kind="ExternalInput").ap()
    wv = nc.dram_tensor("wv", [E, E], BF16, kind="ExternalInput").ap()
    wp = nc.dram_tensor("wp", [E, E], BF16, kind="ExternalInput").ap()
    kcamT_d = nc.dram_tensor("kcamT", [DK, S], BF16, kind="ExternalInput").ap()
    vnat_d = nc.dram_tensor("vnat", [DK, S], BF16, kind="ExternalInput").ap()
    bqv = nc.dram_tensor("bq", [E], F32, kind="ExternalInput").ap()
    gate = nc.dram_tensor("gate", [DK], F32, kind="ExternalInput").ap()
    gateb = nc.dram_tensor("gateb", [DK], F32, kind="ExternalInput").ap()
    lwv = nc.dram_tensor("lw", [DK], F32, kind="ExternalInput").ap()
    bo = nc.dram_tensor("bo", [E], BF16, kind="ExternalInput").ap()
    out = nc.dram_tensor("out", [LOC, E], F32, kind="ExternalOutput").ap()

    with TileContext(nc) as tc:
        with (
            tc.tile_pool(name="const", bufs=1) as pconst,
            tc.tile_pool(name="psS", bufs=1, space="PSUM") as psS,
            tc.tile_pool(name="psB", bufs=1, space="PSUM") as psB,
        ):
            ident_f = pconst.tile([P, P], F32, tag="identf")
            make_identity(nc, ident_f)
            ident = pconst.tile([P, P], BF16, tag="ident")
            nc.vector.tensor_copy(ident, ident_f)
            ones_f = pconst.tile([1, P], F32, tag="onesf")
            nc.vector.memset(ones_f, 1.0)
            ones_b = pconst.tile([1, P], BF16, tag="ones")
            nc.vector.tensor_copy(ones_b, ones_f)
            # consts on gpsimd: keep the sync ring free for weight streams
            gate_sb = pconst.tile([P, 1], F32, tag="gate")
            nc.gpsimd.dma_start(gate_sb, gate.rearrange("(p o) -> p o", o=1))
            gateb_sb = pconst.tile([P, 1], F32, tag="gateb")
            nc.gpsimd.dma_start(gateb_sb, gateb.rearrange("(p o) -> p o", o=1))
            lw_sb = pconst.tile([P, 1], F32, tag="lw")
            nc.gpsimd.dma_start(lw_sb, lwv.rearrange("(p o) -> p o", o=1))
            bq_sb = pconst.tile([P, NT], F32, tag="bq")
            nc.gpsimd.dma_start(bq_sb, bqv.rearrange("(m p) -> p m", p=P))

            def mm(ps, lhsT, rhs, start, stop):
                nc.tensor.matmul(ps, lhsT, rhs, start=start, stop=stop)

            def cpy(i, dst, src):
                # spread psum->sbuf copies across DVE and ACT (GpSimd
                # cannot read PSUM on TRN2)
                if i % 2 == 0:
                    nc.vector.tensor_copy(dst, src)
                else:
                    nc.scalar.activation(dst, src, AF.Copy)

            def slots4():
                return [psS.tile([P, 512], F32, tag="slot", bufs=6,
                                 name=f"sl{j}") for j in range(4)]

            def softmax_exp(pool, slots, rec_out=None, sum_row=None):
                """4 psum slots [128,512] -> exp (bf16 src returned).

                rec_out=None: normalize (exn * 1/rowsum).
                rec_out=AP: leave exp UNNORMALIZED; store 1/rowsum into
                rec_out and the bf16 rowsum ROW (via PE mini-transpose)
                into sum_row [1,128] for the deferred bias matmul."""
                exu = pool.tile([P, S], BF16, tag="exu", bufs=2, name="exu")
                st = [pool.tile([P, 1], F32, tag=f"st{t}", bufs=2,
                                name=f"st{t}") for t in range(4)]
                for t in range(4):
                    nc.scalar.activation(exu[:, ts(t, 512)], slots[t], AF.Exp,
                                         scale=SCALE, accum_out=st[t])
                nc.vector.tensor_tensor(st[0], st[0], st[1], op=ALU.add)
                nc.vector.tensor_tensor(st[2], st[2], st[3], op=ALU.add)
                nc.vector.tensor_tensor(st[0], st[0], st[2], op=ALU.add)
                if rec_out is None:
                    rec = pool.tile([P, 1], F32, tag="rec", bufs=2, name="rec")
                    nc.vector.reciprocal(rec, st[0])
                    exn = pool.tile([P, S], BF16, tag="exn", bufs=2,
                                    name="exn")
                    nc.vector.tensor_scalar_mul(exn, exu, rec)
                    return exn
                nc.vector.reciprocal(rec_out, st[0])
                stb = pool.tile([P, 1], BF16, tag="stb", bufs=2, name="stb")
                nc.vector.tensor_copy(stb, st[0])
                tp = psB.tile([P, 512], BF16, tag="tr", bufs=2, name="tp")
                nc.tensor.matmul(tp[0:1, 0:P], stb, ident, start=True,
                                 stop=True, is_transpose=True,
                                 skip_group_check=True)
                nc.vector.tensor_copy(sum_row, tp[0:1, 0:P])
                return exu

            def softmax_flush(ic, src, dst):
                """16 transposed [j,i]-blocks into dst cols jt*512+ic*128."""
                for jq in range(4):
                    tp = psB.tile([P, 512], BF16, tag="tr", bufs=2, name="tp")
                    for t in range(4):
                        nc.tensor.matmul(
                            tp[:, ts(t, P)], src[:, ts(jq * 4 + t, P)], ident,
                            start=True, stop=True, is_transpose=True,
                            skip_group_check=True)
                    nc.vector.tensor_copy(
                        dst[:, jq * 2048:(jq + 1) * 2048].rearrange(
                            "p (t i) -> p t i", t=4)[:, :, ts(ic, P)],
                        tp.rearrange("p (t i) -> p t i", t=4))

            with tc.tile_pool(name="at", bufs=1) as pat:
                AT = pat.tile([P, NT * 512], BF16, tag="AT")
                rec4 = pat.tile([P, 4], F32, tag="rec4")  # deferred 1/rowsum
                sums_row = pat.tile([1, 4 * P], BF16, tag="sumr")  # rowsums
                with tc.tile_pool(name="cam", bufs=1) as pcam:
                    qcam = pcam.tile([P, LOC], BF16, tag="qcam")
                    kcamT = pcam.tile([P, S], BF16, tag="kcamT")
                    vnat = pcam.tile([P, S], BF16, tag="vnat")
                    bAT = pcam.tile([P, NT * 512], BF16, tag="bAT")
                    T_sb = pcam.tile([P, LOC], BF16, tag="T")


                    with tc.tile_pool(name="hidT", bufs=1) as phid:
                        h = [phid.tile([P, S], BF16, tag=f"h{i}", name=f"h{i}")
                             for i in range(NT)]
                        # s1: load hid^T on scalar/gpsimd only — sync stays
                        # free for the wq stream; local col-chunk first so
                        # s2's first matmuls unblock within ~2us
                        for k in range(NT):
                            eng = (nc.scalar, nc.gpsimd)[k % 2]
                            eng.dma_start(h[k][:, 0:LOC],
                                          hidT_d[ts(k, P), 0:LOC])
                        for k in range(NT):
                            eng = (nc.scalar, nc.gpsimd)[k % 2]
                            eng.dma_start(h[k][:, LOC:S],
                                          hidT_d[ts(k, P), LOC:S])
                        nc.gpsimd.dma_start(kcamT, kcamT_d)
                        nc.gpsimd.dma_start(vnat, vnat_d)

                        with tc.tile_pool(name="ppt", bufs=1) as ppt:
                            pt = [ppt.tile([P, LOC], BF16, tag=f"pt{m}",
                                           name=f"pt{m}") for m in range(NT)]

                            # s2: Q^T (16 x [128,512] bf16)
                            with tc.tile_pool(name="qt", bufs=1) as pqt:
                                qts = [qcam] + [
                                    pqt.tile([P, LOC], BF16, tag=f"qt{m}",
                                             name=f"qt{m}")
                                    for m in range(1, NT)
                                ]
                                for m4 in range(4):
                                    slots = slots4()
                                    for k in range(NT):
                                        wqt = pqt.tile([P, 512], BF16,
                                                       tag="w_in", bufs=6,
                                                       name="wqt")
                                        (nc.sync, nc.scalar)[k % 2].dma_start(
                                            wqt, wq[ts(k, P), ts(m4, 512)])
                                        for j in range(4):
                                            mm(slots[j], wqt[:, ts(j, P)],
                                               h[k][:, 0:LOC],
                                               k == 0, k == NT - 1)
                                    for j in range(4):
                                        m = m4 * 4 + j
                                        if j % 2 == 0:
                                            nc.vector.tensor_scalar_add(
                                                qts[m], slots[j],
                                                bq_sb[:, m:m + 1])
                                        else:
                                            nc.scalar.activation(
                                                qts[m], slots[j], AF.Identity,
                                                bias=bq_sb[:, m:m + 1])

                                # s3: P^T = Wk^T @ Q^T (wkT from host),
                                # interleaved with s5 (base attention ->
                                # baseA^T): each s5 softmax chain hides
                                # under the next dense s3 block
                                def s5_start(ic):
                                    slots = slots4()
                                    for j4 in range(4):
                                        mm(slots[j4], qcam[:, ts(ic, P)],
                                           kcamT[:, ts(j4, 512)], True, True)
                                    return softmax_exp(ppt, slots)

                                s5_src = [None] * 4
                                for m4 in range(4):
                                    slots = slots4()
                                    for k in range(NT):
                                        wkt = pqt.tile([P, 512], BF16,
                                                       tag="w_in", bufs=6,
                                                       name="wkt")
                                        (nc.sync, nc.scalar)[k % 2].dma_start(
                                            wkt, wkT[ts(k, P), ts(m4, 512)])
                                        for j in range(4):
                                            mm(slots[j], wkt[:, ts(j, P)],
                                               qts[k], k == 0, k == NT - 1)
                                    for j in range(4):
                                        cpy(j, pt[m4 * 4 + j], slots[j])
                                    if m4 >= 1:
                                        softmax_flush(m4 - 1, s5_src[m4 - 1],
                                                      bAT)
                                    s5_src[m4] = s5_start(m4)
                                softmax_flush(3, s5_src[3], bAT)

                            # s6: camctx^T + T^T
                            cps = psS.tile([P, 512], F32, tag="slot", bufs=6,
                                           name="cps")
                            for jt in range(NT):
                                mm(cps, vnat[:, ts(jt, P)],
                                   bAT[:, ts(jt, 512)], jt == 0, jt == NT - 1)
                            ttmp = ppt.tile([P, LOC], F32, tag="ttmp", bufs=1)
                            nc.vector.tensor_scalar(
                                ttmp, cps, gate_sb, gateb_sb,
                                op0=ALU.mult, op1=ALU.add)
                            nc.scalar.activation(ttmp, ttmp, AF.Tanh)
                            nc.vector.tensor_scalar_mul(T_sb, ttmp, lw_sb)

                            # s7: main scores -> A^T (SBUF)
                            for ic in range(4):
                                slots = slots4()
                                for k in range(NT):
                                    for j4 in range(4):
                                        mm(slots[j4], pt[k][:, ts(ic, P)],
                                           h[k][:, ts(j4, 512)], k == 0, False)
                                for j4 in range(4):
                                    mm(slots[j4], T_sb[:, ts(ic, P)],
                                       kcamT[:, ts(j4, 512)], False, True)
                                src = softmax_exp(
                                    ppt, slots, rec_out=rec4[:, ic:ic + 1],
                                    sum_row=sums_row[0:1, ts(ic, P)])
                                softmax_flush(ic, src, AT)

                # s8..s10 (hidT/cam freed; AT alive)
                with tc.tile_pool(name="ht", bufs=1) as pht:
                    ht = [pht.tile([P, LOC], BF16, tag=f"ht{m}", name=f"ht{m}")
                          for m in range(NT)]
                    # s8: H^T = hid^T @ A^T
                    for m4 in range(4):
                        slots = slots4()
                        for k in range(NT):
                            hb = pht.tile([P, 512], BF16, tag="w_in",
                                          bufs=6, name="hb")
                            (nc.sync, nc.scalar)[k % 2].dma_start(
                                hb, hid[ts(k, P), ts(m4, 512)])
                            for j in range(4):
                                mm(slots[j], hb[:, ts(j, P)],
                                   AT[:, ts(k, 512)], k == 0, k == NT - 1)
                        for j in range(4):
                            cpy(j, ht[m4 * 4 + j], slots[j])

                    with tc.tile_pool(name="cx", bufs=1) as pcx:
                        cx = [pcx.tile([P, LOC], BF16, tag=f"cx{m}",
                                       name=f"cx{m}") for m in range(NT)]
                        # s9: ctxu^T = Wv^T @ H^T
                        for m4 in range(4):
                            slots = slots4()
                            for k in range(NT):
                                wvt = pcx.tile([P, 512], BF16, tag="w_in",
                                               bufs=6, name="wvt")
                                (nc.sync, nc.scalar)[k % 2].dma_start(
                                    wvt, wv[ts(k, P), ts(m4, 512)])
                                for j in range(4):
                                    mm(slots[j], wvt[:, ts(j, P)], ht[k],
                                       k == 0, k == NT - 1)
                            for j in range(4):
                                cpy(j, cx[m4 * 4 + j], slots[j])

                        # s10: out = rec4 * (ctxu_un @ Wp + rowsum x b_out)
                        # (deferred softmax normalization: the bias enters
                        # PSUM pre-scaled by the rowsum via an outer-product
                        # matmul, so one DVE scale finishes each tile)
                        bo_sb = pcx.tile([1, E], BF16, tag="bo")
                        nc.scalar.dma_start(
                            bo_sb, bo.rearrange("(o f) -> o f", o=1))
                        for n4 in range(4):
                            slots = slots4()
                            for k in range(NT):
                                wpt = pcx.tile([P, 512], BF16, tag="w_in",
                                               bufs=6, name="wpt")
                                (nc.sync, nc.scalar)[k % 2].dma_start(
                                    wpt, wp[ts(k, P), ts(n4, 512)])
                                for ic in range(4):
                                    mm(slots[ic], cx[k][:, ts(ic, P)], wpt,
                                       k == 0, False)
                            for ic in range(4):
                                mm(slots[ic], sums_row[0:1, ts(ic, P)],
                                   bo_sb[0:1, ts(n4, 512)], False, True)
                                ostg = pcx.tile([P, 512], F32, tag="ostg",
                                                bufs=2, name="ostg")
                                nc.vector.tensor_scalar_mul(
                                    ostg, slots[ic], rec4[:, ic:ic + 1])
                                eng = (nc.scalar, nc.gpsimd,
                                       nc.sync)[(n4 * 4 + ic) % 3]
                                eng.dma_start(
                                    out[ts(ic, P), ts(n4, 512)], ostg)

    nc.compile()
    return nc


_NC = None


def _get_nc():
    global _NC
    if _NC is None:
        _NC = build()
    return _NC


def make_in_maps(hidden_states, c_attn_w, c_attn_b, c_proj_w, c_proj_b,
                 cam_gate, cam_w0, cam_w1):
    import ml_dtypes
    BF = ml_dtypes.bfloat16

    hs = np.ascontiguousarray(np.asarray(hidden_states, dtype=np.float32))
    W = np.asarray(c_attn_w, dtype=np.float32)
    b = np.asarray(c_attn_b, dtype=np.float32)
    Wp = np.ascontiguousarray(np.asarray(c_proj_w, dtype=np.float32))
    bp = np.asarray(c_proj_b, dtype=np.float32)
    gate = np.ascontiguousarray(np.asarray(cam_gate, dtype=np.float32))
    w0 = float(np.asarray(cam_w0).reshape(-1)[0])
    w1 = float(np.asarray(cam_w1).reshape(-1)[0])

    wq_b = np.ascontiguousarray(W[:, :E]).astype(BF)
    wkm = W[:, E:2 * E]
    wkT_b = np.ascontiguousarray(wkm.T).astype(BF)
    wvm = W[:, 2 * E:]
    wv_b = np.ascontiguousarray(wvm).astype(BF)
    wp_b = Wp.astype(BF)
    bq = np.ascontiguousarray(b[:E])
    bv = b[2 * E:].astype(np.float64)

    lw = 1.0 / (1.0 + np.exp(-(w0 + w1 * 0.5)))
    lw_arr = np.full(DK, lw, dtype=np.float32)
    gateb = np.ascontiguousarray(gate * b[2 * E:2 * E + DK])
    b_out = (bv @ Wp.astype(np.float64) + bp.astype(np.float64)).astype(BF)

    in_maps = []
    for bi in range(2):
        hb = hs[bi]
        Kc = hb @ wkm[:, :DK]  # K-bias cancels in softmax
        Vc = hb @ wvm[:, :DK]  # V-bias folded into gateb
        for rr in range(4):
            sel = np.concatenate([np.arange(rr * LOC, S),
                                  np.arange(0, rr * LOC)])
            hid_roll = np.ascontiguousarray(hb[sel]).astype(BF)
            hidT_roll = np.ascontiguousarray(hb[sel].T).astype(BF)
            kcamT = np.ascontiguousarray(Kc[sel].T).astype(BF)
            vnat = np.ascontiguousarray(
                Vc[sel].reshape(NT, P, DK).transpose(1, 0, 2).reshape(P, S)
            ).astype(BF)
            in_maps.append({
                "hid": hid_roll, "hidT": hidT_roll, "wq": wq_b, "wkT": wkT_b,
                "wv": wv_b, "wp": wp_b, "kcamT": kcamT, "vnat": vnat,
                "bq": bq, "gate": gate, "gateb": gateb, "lw": lw_arr,
                "bo": b_out,
            })
    return in_maps


def kernel(**inputs):
    nc = _get_nc()
    in_maps = make_in_maps(**inputs)
    res = run_bass_kernel_spmd(nc, in_maps, core_ids=list(range(8)))
    out = np.empty((2, S, E), dtype=np.float32)
    for c in range(8):
        bi, rr = divmod(c, 4)
        out[bi, rr * LOC:(rr + 1) * LOC] = res.results[c]["out"]
    return out
